# revision 35
# baseline (speedup 1.0000x reference)
"""Trainium2 Bass kernel for AstraMambaWrapper (Mamba-1 block over gathered check nodes).

v2 strategy (8 NeuronCores, tensor-parallel over d_inner):
  - Host: gather x_chk = x[seq_idx]; each core gets full x_chk.T (bf16) plus its
    128-channel shard of every weight.
  - The causal depthwise conv is folded into in_proj on the TensorEngine:
    uc = sum_k shift_k(x) @ (W_u * conv_w[:,k]), accumulated in PSUM.
    silu(uc+conv_b) and silu(z) are single ACT ops (silu table).
  - Per 2048-col chunk: x_proj partials -> AllReduce [64, 2048] (8 fine-grained
    ARs, with a tiny warm-up collective at t=0 to absorb ring setup).
  - dt chunk c + scan block c emitted with lag 2 behind in_proj chunks so the
    DVE scan stream starts ~60us in and overlaps the rest of phase A.
  - Selective scan per block: per state n, a = exp(dt*A_n) (ACT),
    b = dtu*B_n (DVE), h = tensor_tensor_scan (DVE), carry via ACT copy,
    ys += C_n*h (c-mult on DVE or GpSimd, accumulation on GpSimd).
  - y = (ys + ua*D_skip)*silu(z); out_proj partials (PE);
    per-block ReduceScatter [2048, 512] -> per-core [256, 512]; LayerNorm on
    ACT (stats via accum) + residual; 8 small RS chunks keep the tail short.
Degenerate-by-construction params (ln_w=1, ln_b=0) are verified on the host
and baked into the graph; all other params are honored from the inputs.
"""

import os
import sys

sys.path.insert(0, "/opt/trn_rl_repo")

import numpy as np
import ml_dtypes

S = 16384
DM = 512
DI = 1024
DS = 16
RK = 32
DC = 4
NCORE = 8
P = DI // NCORE          # 128 channels per core
MC = 2048                # in_proj / AR chunk = scan block length
NQ = S // MC             # 8 chunks/blocks
SROW = S // NCORE        # 2048 output rows per core
SHARE = MC // NCORE      # 256 rows per core per RS chunk
LN_EPS = 1e-5

# phase-C engine split knobs
GPS_YS = False                   # ys accumulation on GpSimd
GPS_C_STATES = 0                 # how many states' c-mult run on GpSimd

BF16 = ml_dtypes.bfloat16

_CACHE = {}


def _build(debug=False):
    import concourse.bass as bass
    import concourse.bacc as bacc
    import concourse.mybir as mybir
    import concourse.tile as tile

    f32 = mybir.dt.float32
    bf16 = mybir.dt.bfloat16
    AF = mybir.ActivationFunctionType
    OP = mybir.AluOpType

    nc = bacc.Bacc("TRN2", target_bir_lowering=False, debug=False, num_devices=NCORE)

    # ---- kernel I/O (per-core shards) ----
    xT = nc.dram_tensor("xT", [DM, S], bf16, kind="ExternalInput")            # x_chk.T
    wuc = nc.dram_tensor("wuc", [128, 4 * DC * P], bf16, kind="ExternalInput")  # [p, kk*DC*P + k*P + d]
    wz = nc.dram_tensor("wz", [128, 4 * P], bf16, kind="ExternalInput")         # [p, kk*P + d]
    wxp = nc.dram_tensor("wxp", [P, RK + 2 * DS], bf16, kind="ExternalInput")
    wdt = nc.dram_tensor("wdt", [RK, P], bf16, kind="ExternalInput")
    wout = nc.dram_tensor("wout", [P, DM], bf16, kind="ExternalInput")
    smallp = nc.dram_tensor("smallp", [P, 4], f32, kind="ExternalInput")      # conv_b, b_dt, D_skip, -conv_b
    alog = nc.dram_tensor("alog", [P, DS], f32, kind="ExternalInput")
    xres = nc.dram_tensor("xres", [SROW, DM], f32, kind="ExternalInput")
    out = nc.dram_tensor("out", [SROW, DM], f32, kind="ExternalOutput")

    # ---- internal DRAM ----
    warm_in = nc.dram_tensor("warm_in", [64, 64], bf16)
    warm_out = nc.dram_tensor("warm_out", [64, 64], bf16, addr_space="Shared")
    dbc_in = [nc.dram_tensor(f"dbc_in{c}", [RK + 2 * DS, MC], bf16) for c in range(NQ)]
    dbc_out = [nc.dram_tensor(f"dbc_out{c}", [RK + 2 * DS, MC], bf16, addr_space="Shared")
               for c in range(NQ)]
    op_in = [nc.dram_tensor(f"op_in{q}", [MC, DM], bf16) for q in range(NQ)]
    # RS at 1024-row granularity: 2 per block, 128 rows per core each
    op_out = [nc.dram_tensor(f"op_out{q}", [128, DM], bf16) for q in range(2 * NQ)]

    if debug:
        dbg_ua = nc.dram_tensor("dbg_ua", [P, S], bf16, kind="ExternalOutput")
        dbg_dt = nc.dram_tensor("dbg_dt", [P, S], bf16, kind="ExternalOutput")
        dbg_zg = nc.dram_tensor("dbg_zg", [P, S], bf16, kind="ExternalOutput")
        dbg_y = nc.dram_tensor("dbg_y", [P, S], bf16, kind="ExternalOutput")

    def bcast_row(src, row):
        """AP reading src[row, :] replicated across 128 partitions."""
        ap = src[row : row + 1, :]
        return bass.AP(ap.tensor, ap.offset, [[0, P]] + list(ap.ap[1:]))

    rg = [list(range(NCORE))]
    xT_r = xT.ap().rearrange("(k p) t -> p k t", p=128)

    with tile.TileContext(nc) as tc:
        with (
            tc.tile_pool(name="const", bufs=1) as cp,
            tc.tile_pool(name="big", bufs=1) as bp,
            tc.tile_pool(name="xkp", bufs=2) as xkp,
            tc.tile_pool(name="work", bufs=2) as wp,
            tc.tile_pool(name="scan", bufs=2) as sp,
            tc.tile_pool(name="ycp", bufs=3) as ycp,
            tc.tile_pool(name="bc", bufs=2) as bcp,
            tc.tile_pool(name="psA", bufs=2, space="PSUM") as psA,
            tc.tile_pool(name="psB", bufs=2, space="PSUM") as psB,
            tc.tile_pool(name="psC", bufs=2, space="PSUM") as psC,
            tc.tile_pool(name="psD", bufs=2, space="PSUM") as psD,
        ):
            # ---- warm-up collective: absorbs first-use ring setup (~40us) ----
            nc.gpsimd.collective_compute(
                "AllReduce", OP.add, replica_groups=rg,
                ins=[warm_in.ap().opt()], outs=[warm_out.ap().opt()])

            # ---- constants to SBUF ----
            wuc_sb = cp.tile([128, 4, DC, P], bf16, tag="wuc")
            nc.sync.dma_start(wuc_sb[:, :, :, :],
                              wuc.ap().rearrange("p (a b d) -> p a b d", a=4, b=DC))
            wz_sb = cp.tile([128, 4, P], bf16, tag="wz")
            nc.sync.dma_start(wz_sb[:, :, :], wz.ap().rearrange("p (a d) -> p a d", a=4))
            wxp_sb = cp.tile([P, RK + 2 * DS], bf16, tag="wxp")
            nc.sync.dma_start(wxp_sb[:, :], wxp[:, :])
            wdt_sb = cp.tile([RK, P], bf16, tag="wdt")
            nc.sync.dma_start(wdt_sb[:, :], wdt[:, :])
            wout_sb = cp.tile([P, DM], bf16, tag="wout")
            nc.sync.dma_start(wout_sb[:, :], wout[:, :])
            smallp_sb = cp.tile([P, 4], f32, tag="smallp")
            nc.sync.dma_start(smallp_sb[:, :], smallp[:, :])
            alog_sb = cp.tile([P, DS], f32, tag="alog")
            nc.sync.dma_start(alog_sb[:, :], alog[:, :])
            A_sb = cp.tile([P, DS], f32, tag="A")
            nc.scalar.activation(A_sb[:, :], alog_sb[:, :], AF.Exp)
            nc.vector.tensor_scalar(A_sb[:, :], A_sb[:, :], -1.0, None, op0=OP.mult)
            carry = cp.tile([P, DS], f32, tag="carry")
            eps_t = cp.tile([P, 1], f32, tag="eps")
            nc.vector.memset(eps_t[:, :], LN_EPS)

            # full-S activations (bf16): 96KB/partition
            ua_sb = bp.tile([P, S], bf16, tag="ua")
            zg_sb = bp.tile([P, S], bf16, tag="zg")
            dt_sb = bp.tile([P, S], bf16, tag="dt")

            # ---- phase A chunk: in_proj (conv folded) + silu + xproj + AR ----
            def emit_inproj(c):
                base = c * MC
                for t4 in range(MC // 512):
                    lo = base + t4 * 512
                    xk = xkp.tile([128, 4, 515], bf16, tag="xk")
                    if lo == 0:
                        nc.vector.memset(xk[:, :, 0:3], 0.0)
                        nc.sync.dma_start(xk[:, :, 3:515], xT_r[:, :, 0:512])
                    else:
                        nc.sync.dma_start(xk[:, :, :], xT_r[:, :, lo - 3 : lo + 512])
                    pu = psA.tile([P, 512], f32, tag="pu")
                    for kk in range(4):
                        for k in range(DC):
                            nc.tensor.matmul(pu[:, :], lhsT=wuc_sb[:, kk, k, :],
                                             rhs=xk[:, kk, k : k + 512],
                                             start=(kk == 0 and k == 0),
                                             stop=(kk == 3 and k == DC - 1))
                    pz = psB.tile([P, 512], f32, tag="pz")
                    for kk in range(4):
                        nc.tensor.matmul(pz[:, :], lhsT=wz_sb[:, kk, :],
                                         rhs=xk[:, kk, 3:515],
                                         start=(kk == 0), stop=(kk == 3))
                    # silu via exp/ln/exp (keeps ACT on one table):
                    # sigma(v) = exp(-ln(1+exp(-v)))
                    e1 = wp.tile([P, 512], bf16, tag="e1")
                    nc.scalar.activation(e1[:, :], pu[:, :], AF.Exp, scale=-1.0,
                                         bias=smallp_sb[:, 3:4])
                    l1 = wp.tile([P, 512], bf16, tag="l1")
                    nc.scalar.activation(l1[:, :], e1[:, :], AF.Ln, bias=1.0)
                    s1 = wp.tile([P, 512], bf16, tag="e1")
                    nc.scalar.activation(s1[:, :], l1[:, :], AF.Exp, scale=-1.0)
                    nc.vector.scalar_tensor_tensor(ua_sb[:, lo : lo + 512], pu[:, :],
                                                   smallp_sb[:, 0:1], s1[:, :],
                                                   op0=OP.add, op1=OP.mult)
                    # stage raw z; silu(z) is deferred to the block epilogue
                    # to keep phase-A ACT pressure low
                    nc.scalar.activation(zg_sb[:, lo : lo + 512], pz[:, :], AF.Copy)
                    pd = psC.tile([RK + 2 * DS, 512], f32, tag="pd")
                    nc.tensor.matmul(pd[:, :], lhsT=wxp_sb[:, :],
                                     rhs=ua_sb[:, lo : lo + 512], start=True, stop=True)
                    de = wp.tile([RK + 2 * DS, 512], bf16, tag="de")
                    nc.scalar.activation(de[:, :], pd[:, :], AF.Copy)
                    nc.sync.dma_start(dbc_in[c][:, t4 * 512 : t4 * 512 + 512], de[:, :])
                nc.gpsimd.collective_compute(
                    "AllReduce", OP.add, replica_groups=rg,
                    ins=[dbc_in[c].ap().opt()], outs=[dbc_out[c].ap().opt()])

            # ---- dt chunk: softplus(dt_low @ W_dt + b_dt) ----
            def emit_dt(c):
                for t4 in range(MC // 512):
                    lo = c * MC + t4 * 512
                    dl = wp.tile([RK, 512], bf16, tag="dl")
                    nc.sync.dma_start(dl[:, :], dbc_out[c][0:RK, t4 * 512 : t4 * 512 + 512])
                    pt = psC.tile([P, 512], f32, tag="pd")
                    nc.tensor.matmul(pt[:, :], lhsT=wdt_sb[:, :], rhs=dl[:, :],
                                     start=True, stop=True)
                    ex = wp.tile([P, 512], bf16, tag="de")
                    nc.scalar.activation(ex[:, :], pt[:, :], AF.Exp,
                                         bias=smallp_sb[:, 1:2])
                    nc.scalar.activation(dt_sb[:, lo : lo + 512], ex[:, :], AF.Ln,
                                         bias=1.0)

            # ---- LayerNorm + residual for RS chunk q (one 128-row tile) ----
            def emit_ln(q):
                for st in range(1):
                    lo = q * 128
                    yt = wp.tile([128, DM], bf16, tag="yt", name=f"yt_{q}_{st}")
                    nc.sync.dma_start(yt[:, :], op_out[q][:, :])
                    musum = wp.tile([128, 1], f32, tag="mus", name=f"mus_{q}_{st}")
                    dump2 = wp.tile([128, DM], bf16, tag="ob", name=f"dm2_{q}_{st}")
                    nc.scalar.activation(dump2[:, :], yt[:, :], AF.Copy,
                                         accum_out=musum[:, :])
                    dump = wp.tile([128, DM], bf16, tag="yt", name=f"dmp_{q}_{st}")
                    sqsum = wp.tile([128, 1], f32, tag="sqs", name=f"sqs_{q}_{st}")
                    nc.scalar.activation(dump[:, :], yt[:, :], AF.Square,
                                         accum_out=sqsum[:, :])
                    mu_neg = wp.tile([128, 1], f32, tag="mun", name=f"mun_{q}_{st}")
                    nc.vector.tensor_scalar(mu_neg[:, :], musum[:, :], -1.0 / DM, None,
                                            op0=OP.mult)
                    mu2 = wp.tile([128, 1], f32, tag="mu2", name=f"mu2_{q}_{st}")
                    nc.vector.tensor_tensor(mu2[:, :], mu_neg[:, :], mu_neg[:, :],
                                            op=OP.mult)
                    var = wp.tile([128, 1], f32, tag="var", name=f"var_{q}_{st}")
                    nc.vector.scalar_tensor_tensor(var[:, :], sqsum[:, :], 1.0 / DM,
                                                   mu2[:, :], op0=OP.mult,
                                                   op1=OP.subtract)
                    # rstd = exp(-0.5*ln(var+eps))
                    lv = wp.tile([128, 1], f32, tag="lv", name=f"lv_{q}_{st}")
                    nc.scalar.activation(lv[:, :], var[:, :], AF.Ln,
                                         bias=eps_t[:, 0:1])
                    rstd = wp.tile([128, 1], f32, tag="rst", name=f"rst_{q}_{st}")
                    nc.scalar.activation(rstd[:, :], lv[:, :], AF.Exp, scale=-0.5)
                    nb = wp.tile([128, 1], f32, tag="nb", name=f"nb_{q}_{st}")
                    nc.vector.tensor_tensor(nb[:, :], mu_neg[:, :], rstd[:, :],
                                            op=OP.mult)
                    normed = wp.tile([128, DM], f32, tag="dl", name=f"nrm_{q}_{st}")
                    nc.scalar.activation(normed[:, :], yt[:, :], AF.Identity,
                                         scale=rstd[:, 0:1], bias=nb[:, 0:1])
                    xr = wp.tile([128, DM], f32, tag="xr", name=f"xr_{q}_{st}")
                    nc.sync.dma_start(xr[:, :], xres[lo : lo + 128, :])
                    of = wp.tile([128, DM], f32, tag="xr", name=f"of_{q}_{st}")
                    nc.vector.tensor_tensor(of[:, :], normed[:, :], xr[:, :], op=OP.add)
                    nc.sync.dma_start(out[lo : lo + 128, :], of[:, :])

            # ---- scan block b (2048 timesteps) ----
            def emit_scan(b, dt_for=None):
                lo = b * MC
                dt_blk = dt_sb[:, lo : lo + MC]
                # deferred silu(z): ACT chain emitted first so it completes
                # long before the epilogue needs zgb
                z_blk = zg_sb[:, lo : lo + MC]
                ez = sp.tile([P, MC], bf16, tag="a")
                nc.scalar.activation(ez[:, :], z_blk, AF.Exp, scale=-1.0)
                lz = sp.tile([P, MC], bf16, tag="dtu")
                nc.scalar.activation(lz[:, :], ez[:, :], AF.Ln, bias=1.0)
                sz = sp.tile([P, MC], bf16, tag="zs")
                nc.scalar.activation(sz[:, :], lz[:, :], AF.Exp, scale=-1.0)
                dtu = sp.tile([P, MC], bf16, tag="dtu")
                nc.vector.tensor_tensor(dtu[:, :], dt_blk, ua_sb[:, lo : lo + MC],
                                        op=OP.mult)
                ys = [None, None]
                for n in range(DS):
                    # dt for the next block is emitted mid-block: late enough
                    # that its AllReduce has completed (no ACT-queue stall),
                    # early enough to be ready at the next block start.
                    if n == 4 and dt_for is not None:
                        emit_dt(dt_for)
                    a_t = sp.tile([P, MC], bf16, tag="a")
                    nc.scalar.activation(a_t[:, :], dt_blk, AF.Exp,
                                         scale=A_sb[:, n : n + 1])
                    bbc = bcp.tile([P, MC], bf16, tag="bbc")
                    nc.sync.dma_start(bbc[:, :], bcast_row(dbc_out[b], RK + n))
                    b_t = sp.tile([P, MC], bf16, tag="b")
                    nc.vector.tensor_tensor(b_t[:, :], dtu[:, :], bbc[:, :], op=OP.mult)
                    h_t = sp.tile([P, MC], bf16, tag="h")
                    init = 0.0 if b == 0 else carry[:, n : n + 1]
                    nc.vector.tensor_tensor_scan(h_t[:, :], a_t[:, :], b_t[:, :],
                                                 initial=init, op0=OP.mult, op1=OP.add)
                    if b < NQ - 1:
                        nc.scalar.activation(carry[:, n : n + 1],
                                             h_t[:, MC - 1 : MC], AF.Copy)
                    cbc = bcp.tile([P, MC], bf16, tag="cbc")
                    nc.sync.dma_start(cbc[:, :], bcast_row(dbc_out[b], RK + DS + n))
                    yc = ycp.tile([P, MC], bf16, tag="yc")
                    nc.vector.tensor_tensor(yc[:, :], h_t[:, :], cbc[:, :],
                                            op=OP.mult)
                    # ys accumulation on DMA engines: two independent chains
                    # (even/odd states) so RMW links overlap
                    ch = n % 2
                    if ys[ch] is None:
                        ys[ch] = sp.tile([P, MC], bf16, tag="ys",
                                         name=f"ys{ch}_{b}")
                        nc.gpsimd.dma_start(ys[ch][:, :], yc[:, :])
                    else:
                        nc.gpsimd.dma_start(ys[ch][:, :], yc[:, :],
                                            accum_op=OP.add)
                zgb = sp.tile([P, MC], bf16, tag="zs")
                nc.vector.tensor_tensor(zgb[:, :], z_blk, sz[:, :], op=OP.mult)
                # epilogue: y = (ys_e + ys_o + ua*D_skip) * silu(z)
                skip = sp.tile([P, MC], bf16, tag="b")
                nc.vector.tensor_scalar(skip[:, :], ua_sb[:, lo : lo + MC],
                                        smallp_sb[:, 2:3], None, op0=OP.mult)
                nc.gpsimd.dma_start(ys[0][:, :], skip[:, :], accum_op=OP.add)
                tot = sp.tile([P, MC], bf16, tag="a")
                nc.vector.tensor_tensor(tot[:, :], ys[0][:, :], ys[1][:, :],
                                        op=OP.add)
                y_t = sp.tile([P, MC], bf16, tag="b")
                nc.vector.tensor_tensor(y_t[:, :], tot[:, :], zgb[:, :], op=OP.mult)
                if debug:
                    nc.sync.dma_start(dbg_y[:, lo : lo + MC], y_t[:, :])
                # out_proj partials for this block; RS per 1024-row half
                for st in range(MC // 128):
                    po = psD.tile([128, DM], f32, tag="po")
                    nc.tensor.matmul(po[:, :], lhsT=y_t[:, st * 128 : (st + 1) * 128],
                                     rhs=wout_sb[:, :], start=True, stop=True)
                    ob = wp.tile([128, DM], bf16, tag="ob")
                    nc.scalar.activation(ob[:, :], po[:, :], AF.Copy)
                    nc.sync.dma_start(op_in[b][st * 128 : (st + 1) * 128, :], ob[:, :])
                    if st % 8 == 7:
                        half = st // 8
                        q = 2 * b + half
                        nc.gpsimd.collective_compute(
                            "ReduceScatter", OP.add, replica_groups=rg,
                            ins=[op_in[b][half * 1024 : (half + 1) * 1024, :].opt()],
                            outs=[op_out[q].ap().opt()])
                        emit_ln(q)

            # ---- main schedule: lag-2 pipeline; dt(b+1) emitted mid-scan(b) ----
            for c in range(NQ):
                if c >= 2:
                    emit_scan(c - 2, dt_for=c - 1)
                emit_inproj(c)
                if c == 1:
                    emit_dt(0)
            emit_scan(NQ - 2, dt_for=NQ - 1)
            emit_scan(NQ - 1)

            if debug:
                nc.sync.dma_start(dbg_ua[:, :], ua_sb[:, :])
                nc.sync.dma_start(dbg_dt[:, :], dt_sb[:, :])
                nc.sync.dma_start(dbg_zg[:, :], zg_sb[:, :])

    # Restrict ACT tables to the two we use so the load-insertion pass
    # inserts switches only at clean phase boundaries.
    import concourse.bacc as bacc_mod
    orig_tables = bacc_mod.get_activation_tables
    KEEP = ("natural_log_exp_and_others",)

    def _two_tables(arch):
        t = orig_tables(arch)
        return {k: (v if k in KEEP else set()) for k, v in t.items()}

    bacc_mod.get_activation_tables = _two_tables
    try:
        nc.compile()
    finally:
        bacc_mod.get_activation_tables = orig_tables
    return nc


def _get_nc():
    if "nc" not in _CACHE:
        _CACHE["nc"] = _build()
    return _CACHE["nc"]


def _core_rows(i):
    """Absolute check-node indices held by core i's output, in output order."""
    return np.concatenate(
        [np.arange(q * 1024 + i * 128, q * 1024 + (i + 1) * 128)
         for q in range(2 * NQ)])


def _make_in_maps(inputs):
    x = np.ascontiguousarray(np.asarray(inputs["x"], dtype=np.float32))
    seq_idx = np.asarray(inputs["seq_idx"], dtype=np.int64)
    W_in = np.asarray(inputs["W_in"], dtype=np.float32)
    conv_w = np.asarray(inputs["conv_w"], dtype=np.float32)
    conv_b = np.asarray(inputs["conv_b"], dtype=np.float32)
    W_xproj = np.asarray(inputs["W_xproj"], dtype=np.float32)
    W_dt = np.asarray(inputs["W_dt"], dtype=np.float32)
    b_dt = np.asarray(inputs["b_dt"], dtype=np.float32)
    A_log = np.asarray(inputs["A_log"], dtype=np.float32)
    D_skip = np.asarray(inputs["D_skip"], dtype=np.float32)
    W_out = np.asarray(inputs["W_out"], dtype=np.float32)
    ln_w = np.asarray(inputs["ln_w"], dtype=np.float32)
    ln_b = np.asarray(inputs["ln_b"], dtype=np.float32)

    # ln scale/bias are identity by construction; they are baked into the graph.
    assert np.allclose(ln_w, 1.0) and np.allclose(ln_b, 0.0), "non-identity LN params unsupported"

    x_chk = x[seq_idx]                               # [S, DM]
    xT = np.ascontiguousarray(x_chk.T).astype(BF16)  # [DM, S]

    in_maps = []
    for i in range(NCORE):
        cs = slice(i * P, (i + 1) * P)
        # conv folded into in_proj: W_k[:, d] = W_in[:, cs][:, d] * conv_w[cs][d, k]
        Wu = W_in[:, cs]                                       # [DM, P]
        cw = conv_w[cs]                                        # [P, DC]
        # wuc layout [p, kk, k, d]: p = dm % 128, kk = dm // 128
        wuc = np.empty((128, 4, DC, P), np.float32)
        for kk in range(4):
            blk = Wu[kk * 128 : (kk + 1) * 128, :]             # [128, P]
            for k in range(DC):
                wuc[:, kk, k, :] = blk * cw[None, :, k]
        wz = np.ascontiguousarray(
            W_in[:, DI + i * P : DI + (i + 1) * P].reshape(4, 128, P).transpose(1, 0, 2))
        in_maps.append({
            "xT": xT,
            "wuc": np.ascontiguousarray(wuc.reshape(128, 4 * DC * P)).astype(BF16),
            "wz": np.ascontiguousarray(wz.reshape(128, 4 * P)).astype(BF16),
            "wxp": np.ascontiguousarray(W_xproj[cs]).astype(BF16),
            "wdt": np.ascontiguousarray(W_dt[:, cs]).astype(BF16),
            "wout": np.ascontiguousarray(W_out[cs]).astype(BF16),
            "smallp": np.ascontiguousarray(
                np.stack([conv_b[cs], b_dt[cs], D_skip[cs], -conv_b[cs]],
                         axis=1).astype(np.float32)),
            "alog": np.ascontiguousarray(A_log[cs]),
            "xres": np.ascontiguousarray(x_chk[_core_rows(i)]),
        })
    return x, seq_idx, in_maps


def kernel(**inputs):
    from concourse.bass_utils import run_bass_kernel_spmd

    x, seq_idx, in_maps = _make_in_maps(inputs)
    nc = _get_nc()
    trace = bool(int(os.environ.get("KERNEL_TRACE", "0")))
    res = run_bass_kernel_spmd(nc, in_maps, core_ids=list(range(NCORE)), trace=trace)
    if trace:
        _CACHE["last_exec_time_ns"] = res.exec_time_ns
        _CACHE["last_results"] = res
    y = np.empty((S, DM), np.float32)
    for i in range(NCORE):
        y[_core_rows(i)] = np.asarray(res.results[i]["out"])
    outp = x.copy()
    outp[seq_idx] = y
    return outp


# revision 40
# speedup vs baseline: 1.1250x; 1.1250x over previous
"""Trainium2 Bass kernel for AstraMambaWrapper (Mamba-1 block over gathered check nodes).

v2 strategy (8 NeuronCores, tensor-parallel over d_inner):
  - Host: gather x_chk = x[seq_idx]; each core gets full x_chk.T (bf16) plus its
    128-channel shard of every weight.
  - The causal depthwise conv is folded into in_proj on the TensorEngine:
    uc = sum_k shift_k(x) @ (W_u * conv_w[:,k]), accumulated in PSUM.
    silu(uc+conv_b) and silu(z) are single ACT ops (silu table).
  - Per 2048-col chunk: x_proj partials -> AllReduce [64, 2048] (8 fine-grained
    ARs, with a tiny warm-up collective at t=0 to absorb ring setup).
  - dt chunk c + scan block c emitted with lag 2 behind in_proj chunks so the
    DVE scan stream starts ~60us in and overlaps the rest of phase A.
  - Selective scan per block: per state n, a = exp(dt*A_n) (ACT),
    b = dtu*B_n (DVE), h = tensor_tensor_scan (DVE), carry via ACT copy,
    ys += C_n*h (c-mult on DVE or GpSimd, accumulation on GpSimd).
  - y = (ys + ua*D_skip)*silu(z); out_proj partials (PE);
    per-block ReduceScatter [2048, 512] -> per-core [256, 512]; LayerNorm on
    ACT (stats via accum) + residual; 8 small RS chunks keep the tail short.
Degenerate-by-construction params (ln_w=1, ln_b=0) are verified on the host
and baked into the graph; all other params are honored from the inputs.
"""

import os
import sys

sys.path.insert(0, "/opt/trn_rl_repo")

import numpy as np
import ml_dtypes

S = 16384
DM = 512
DI = 1024
DS = 16
RK = 32
DC = 4
NCORE = 8
P = DI // NCORE          # 128 channels per core
MC = 2048                # in_proj / AR chunk = scan block length
NQ = S // MC             # 8 chunks/blocks
SROW = S // NCORE        # 2048 output rows per core
SHARE = MC // NCORE      # 256 rows per core per RS chunk
LN_EPS = 1e-5

# phase-C engine split knobs
GPS_YS = False                   # ys accumulation on GpSimd
GPS_C_STATES = 0                 # how many states' c-mult run on GpSimd

BF16 = ml_dtypes.bfloat16

_CACHE = {}


def _build(debug=False):
    import concourse.bass as bass
    import concourse.bacc as bacc
    import concourse.mybir as mybir
    import concourse.tile as tile

    f32 = mybir.dt.float32
    bf16 = mybir.dt.bfloat16
    AF = mybir.ActivationFunctionType
    OP = mybir.AluOpType

    nc = bacc.Bacc("TRN2", target_bir_lowering=False, debug=False, num_devices=NCORE)

    # ---- kernel I/O (per-core shards) ----
    xT = nc.dram_tensor("xT", [DM, S], bf16, kind="ExternalInput")            # x_chk.T
    wuc = nc.dram_tensor("wuc", [128, 4 * DC * P], bf16, kind="ExternalInput")  # [p, kk*DC*P + k*P + d]
    wz = nc.dram_tensor("wz", [128, 4 * P], bf16, kind="ExternalInput")         # [p, kk*P + d]
    wxp = nc.dram_tensor("wxp", [P, RK + 2 * DS], bf16, kind="ExternalInput")
    wdt = nc.dram_tensor("wdt", [RK, P], bf16, kind="ExternalInput")
    wout = nc.dram_tensor("wout", [P, DM], bf16, kind="ExternalInput")
    smallp = nc.dram_tensor("smallp", [P, 4], f32, kind="ExternalInput")      # conv_b, b_dt, D_skip, -conv_b
    alog = nc.dram_tensor("alog", [P, DS], f32, kind="ExternalInput")
    xres = nc.dram_tensor("xres", [SROW, DM], f32, kind="ExternalInput")
    out = nc.dram_tensor("out", [SROW, DM], f32, kind="ExternalOutput")

    # ---- internal DRAM ----
    warm_in = nc.dram_tensor("warm_in", [64, 64], bf16)
    warm_out = nc.dram_tensor("warm_out", [64, 64], bf16, addr_space="Shared")
    dbc_in = [nc.dram_tensor(f"dbc_in{c}", [RK + 2 * DS, MC], bf16) for c in range(NQ)]
    dbc_out = [nc.dram_tensor(f"dbc_out{c}", [RK + 2 * DS, MC], bf16, addr_space="Shared")
               for c in range(NQ)]
    op_in = [nc.dram_tensor(f"op_in{q}", [MC, DM], bf16) for q in range(NQ)]
    # RS at 1024-row granularity: 2 per block, 128 rows per core each
    op_out = [nc.dram_tensor(f"op_out{q}", [128, DM], bf16) for q in range(2 * NQ)]

    if debug:
        dbg_ua = nc.dram_tensor("dbg_ua", [P, S], bf16, kind="ExternalOutput")
        dbg_dt = nc.dram_tensor("dbg_dt", [P, S], bf16, kind="ExternalOutput")
        dbg_zg = nc.dram_tensor("dbg_zg", [P, S], bf16, kind="ExternalOutput")
        dbg_y = nc.dram_tensor("dbg_y", [P, S], bf16, kind="ExternalOutput")

    def bcast_row(src, row):
        """AP reading src[row, :] replicated across 128 partitions."""
        ap = src[row : row + 1, :]
        return bass.AP(ap.tensor, ap.offset, [[0, P]] + list(ap.ap[1:]))

    rg = [list(range(NCORE))]
    xT_r = xT.ap().rearrange("(k p) t -> p k t", p=128)

    with tile.TileContext(nc) as tc:
        with (
            tc.tile_pool(name="const", bufs=1) as cp,
            tc.tile_pool(name="big", bufs=1) as bp,
            tc.tile_pool(name="xkp", bufs=2) as xkp,
            tc.tile_pool(name="work", bufs=2) as wp,
            tc.tile_pool(name="scan", bufs=2) as sp,
            tc.tile_pool(name="ycp", bufs=3) as ycp,
            tc.tile_pool(name="bc", bufs=2) as bcp,
            tc.tile_pool(name="psA", bufs=2, space="PSUM") as psA,
            tc.tile_pool(name="psB", bufs=2, space="PSUM") as psB,
            tc.tile_pool(name="psC", bufs=2, space="PSUM") as psC,
            tc.tile_pool(name="psD", bufs=2, space="PSUM") as psD,
        ):
            # ---- warm-up collective: absorbs first-use ring setup (~40us) ----
            nc.gpsimd.collective_compute(
                "AllReduce", OP.add, replica_groups=rg,
                ins=[warm_in.ap().opt()], outs=[warm_out.ap().opt()])

            # ---- constants to SBUF ----
            wuc_sb = cp.tile([128, 4, DC, P], bf16, tag="wuc")
            nc.sync.dma_start(wuc_sb[:, :, :, :],
                              wuc.ap().rearrange("p (a b d) -> p a b d", a=4, b=DC))
            wz_sb = cp.tile([128, 4, P], bf16, tag="wz")
            nc.sync.dma_start(wz_sb[:, :, :], wz.ap().rearrange("p (a d) -> p a d", a=4))
            wxp_sb = cp.tile([P, RK + 2 * DS], bf16, tag="wxp")
            nc.sync.dma_start(wxp_sb[:, :], wxp[:, :])
            wdt_sb = cp.tile([RK, P], bf16, tag="wdt")
            nc.sync.dma_start(wdt_sb[:, :], wdt[:, :])
            wout_sb = cp.tile([P, DM], bf16, tag="wout")
            nc.sync.dma_start(wout_sb[:, :], wout[:, :])
            smallp_sb = cp.tile([P, 4], f32, tag="smallp")
            nc.sync.dma_start(smallp_sb[:, :], smallp[:, :])
            alog_sb = cp.tile([P, DS], f32, tag="alog")
            nc.sync.dma_start(alog_sb[:, :], alog[:, :])
            A_sb = cp.tile([P, DS], f32, tag="A")
            nc.scalar.activation(A_sb[:, :], alog_sb[:, :], AF.Exp)
            nc.vector.tensor_scalar(A_sb[:, :], A_sb[:, :], -1.0, None, op0=OP.mult)
            carry = cp.tile([P, DS], f32, tag="carry")
            eps_t = cp.tile([P, 1], f32, tag="eps")
            nc.vector.memset(eps_t[:, :], LN_EPS)

            # full-S activations (bf16): 96KB/partition
            ua_sb = bp.tile([P, S], bf16, tag="ua")
            zg_sb = bp.tile([P, S], bf16, tag="zg")
            dt_sb = bp.tile([P, S], bf16, tag="dt")

            # ---- phase A chunk: in_proj (conv folded) + silu + xproj + AR ----
            def emit_inproj(c):
                base = c * MC
                for t4 in range(MC // 512):
                    lo = base + t4 * 512
                    xk = xkp.tile([128, 4, 515], bf16, tag="xk")
                    if lo == 0:
                        nc.vector.memset(xk[:, :, 0:3], 0.0)
                        nc.sync.dma_start(xk[:, :, 3:515], xT_r[:, :, 0:512])
                    else:
                        nc.sync.dma_start(xk[:, :, :], xT_r[:, :, lo - 3 : lo + 512])
                    pu = psA.tile([P, 512], f32, tag="pu")
                    for kk in range(4):
                        for k in range(DC):
                            nc.tensor.matmul(pu[:, :], lhsT=wuc_sb[:, kk, k, :],
                                             rhs=xk[:, kk, k : k + 512],
                                             start=(kk == 0 and k == 0),
                                             stop=(kk == 3 and k == DC - 1))
                    pz = psB.tile([P, 512], f32, tag="pz")
                    for kk in range(4):
                        nc.tensor.matmul(pz[:, :], lhsT=wz_sb[:, kk, :],
                                         rhs=xk[:, kk, 3:515],
                                         start=(kk == 0), stop=(kk == 3))
                    nc.scalar.activation(ua_sb[:, lo : lo + 512], pu[:, :], AF.Silu,
                                         bias=smallp_sb[:, 0:1])
                    nc.scalar.activation(zg_sb[:, lo : lo + 512], pz[:, :], AF.Silu)
                    pd = psC.tile([RK + 2 * DS, 512], f32, tag="pd")
                    nc.tensor.matmul(pd[:, :], lhsT=wxp_sb[:, :],
                                     rhs=ua_sb[:, lo : lo + 512], start=True, stop=True)
                    de = wp.tile([RK + 2 * DS, 512], bf16, tag="de")
                    nc.scalar.activation(de[:, :], pd[:, :], AF.Copy)
                    nc.sync.dma_start(dbc_in[c][:, t4 * 512 : t4 * 512 + 512], de[:, :])
                nc.gpsimd.collective_compute(
                    "AllReduce", OP.add, replica_groups=rg,
                    ins=[dbc_in[c].ap().opt()], outs=[dbc_out[c].ap().opt()])

            # ---- dt chunk: softplus(dt_low @ W_dt + b_dt) ----
            def emit_dt(c):
                for t4 in range(MC // 512):
                    lo = c * MC + t4 * 512
                    dl = wp.tile([RK, 512], bf16, tag="dl")
                    nc.sync.dma_start(dl[:, :], dbc_out[c][0:RK, t4 * 512 : t4 * 512 + 512])
                    pt = psC.tile([P, 512], f32, tag="pd")
                    nc.tensor.matmul(pt[:, :], lhsT=wdt_sb[:, :], rhs=dl[:, :],
                                     start=True, stop=True)
                    ex = wp.tile([P, 512], bf16, tag="de")
                    nc.scalar.activation(ex[:, :], pt[:, :], AF.Exp,
                                         bias=smallp_sb[:, 1:2])
                    nc.scalar.activation(dt_sb[:, lo : lo + 512], ex[:, :], AF.Ln,
                                         bias=1.0)

            # ---- LayerNorm + residual for RS chunk q (one 128-row tile) ----
            def emit_ln(q):
                for st in range(1):
                    lo = q * 128
                    yt = wp.tile([128, DM], bf16, tag="yt", name=f"yt_{q}_{st}")
                    nc.sync.dma_start(yt[:, :], op_out[q][:, :])
                    musum = wp.tile([128, 1], f32, tag="mus", name=f"mus_{q}_{st}")
                    dump2 = wp.tile([128, DM], bf16, tag="ob", name=f"dm2_{q}_{st}")
                    nc.scalar.activation(dump2[:, :], yt[:, :], AF.Copy,
                                         accum_out=musum[:, :])
                    dump = wp.tile([128, DM], bf16, tag="yt", name=f"dmp_{q}_{st}")
                    sqsum = wp.tile([128, 1], f32, tag="sqs", name=f"sqs_{q}_{st}")
                    nc.scalar.activation(dump[:, :], yt[:, :], AF.Square,
                                         accum_out=sqsum[:, :])
                    mu_neg = wp.tile([128, 1], f32, tag="mun", name=f"mun_{q}_{st}")
                    nc.vector.tensor_scalar(mu_neg[:, :], musum[:, :], -1.0 / DM, None,
                                            op0=OP.mult)
                    mu2 = wp.tile([128, 1], f32, tag="mu2", name=f"mu2_{q}_{st}")
                    nc.vector.tensor_tensor(mu2[:, :], mu_neg[:, :], mu_neg[:, :],
                                            op=OP.mult)
                    var = wp.tile([128, 1], f32, tag="var", name=f"var_{q}_{st}")
                    nc.vector.scalar_tensor_tensor(var[:, :], sqsum[:, :], 1.0 / DM,
                                                   mu2[:, :], op0=OP.mult,
                                                   op1=OP.subtract)
                    # rstd = exp(-0.5*ln(var+eps))
                    lv = wp.tile([128, 1], f32, tag="lv", name=f"lv_{q}_{st}")
                    nc.scalar.activation(lv[:, :], var[:, :], AF.Ln,
                                         bias=eps_t[:, 0:1])
                    rstd = wp.tile([128, 1], f32, tag="rst", name=f"rst_{q}_{st}")
                    nc.scalar.activation(rstd[:, :], lv[:, :], AF.Exp, scale=-0.5)
                    nb = wp.tile([128, 1], f32, tag="nb", name=f"nb_{q}_{st}")
                    nc.vector.tensor_tensor(nb[:, :], mu_neg[:, :], rstd[:, :],
                                            op=OP.mult)
                    normed = wp.tile([128, DM], f32, tag="dl", name=f"nrm_{q}_{st}")
                    nc.scalar.activation(normed[:, :], yt[:, :], AF.Identity,
                                         scale=rstd[:, 0:1], bias=nb[:, 0:1])
                    xr = wp.tile([128, DM], f32, tag="xr", name=f"xr_{q}_{st}")
                    nc.sync.dma_start(xr[:, :], xres[lo : lo + 128, :])
                    of = wp.tile([128, DM], f32, tag="xr", name=f"of_{q}_{st}")
                    nc.vector.tensor_tensor(of[:, :], normed[:, :], xr[:, :], op=OP.add)
                    nc.sync.dma_start(out[lo : lo + 128, :], of[:, :])

            # ---- scan block b (2048 timesteps) ----
            def emit_scan(b, dt_for=None):
                lo = b * MC
                dt_blk = dt_sb[:, lo : lo + MC]
                dtu = sp.tile([P, MC], bf16, tag="dtu")
                nc.vector.tensor_tensor(dtu[:, :], dt_blk, ua_sb[:, lo : lo + MC],
                                        op=OP.mult)
                ys = None
                for n in range(DS):
                    # dt for the next block is emitted mid-block: late enough
                    # that its AllReduce has completed (no ACT-queue stall),
                    # early enough to be ready at the next block start.
                    if n == 4 and dt_for is not None:
                        emit_dt(dt_for)
                    a_t = sp.tile([P, MC], bf16, tag="a")
                    nc.scalar.activation(a_t[:, :], dt_blk, AF.Exp,
                                         scale=A_sb[:, n : n + 1])
                    bbc = bcp.tile([P, MC], bf16, tag="bbc")
                    nc.sync.dma_start(bbc[:, :], bcast_row(dbc_out[b], RK + n))
                    b_t = sp.tile([P, MC], bf16, tag="b")
                    nc.vector.tensor_tensor(b_t[:, :], dtu[:, :], bbc[:, :], op=OP.mult)
                    h_t = sp.tile([P, MC], bf16, tag="h")
                    init = 0.0 if b == 0 else carry[:, n : n + 1]
                    nc.vector.tensor_tensor_scan(h_t[:, :], a_t[:, :], b_t[:, :],
                                                 initial=init, op0=OP.mult, op1=OP.add)
                    if b < NQ - 1:
                        nc.scalar.activation(carry[:, n : n + 1],
                                             h_t[:, MC - 1 : MC], AF.Copy)
                    cbc = bcp.tile([P, MC], bf16, tag="cbc")
                    nc.sync.dma_start(cbc[:, :], bcast_row(dbc_out[b], RK + DS + n))
                    yc = ycp.tile([P, MC], bf16, tag="yc")
                    nc.vector.tensor_tensor(yc[:, :], h_t[:, :], cbc[:, :],
                                            op=OP.mult)
                    if ys is None:
                        ys = yc
                    else:
                        ys2 = sp.tile([P, MC], bf16, tag="ys")
                        nc.vector.tensor_tensor(ys2[:, :], ys[:, :], yc[:, :],
                                                op=OP.add)
                        ys = ys2
                # epilogue: y = (ys + ua*D_skip) * silu(z)
                skip = sp.tile([P, MC], bf16, tag="b")
                nc.vector.tensor_scalar(skip[:, :], ua_sb[:, lo : lo + MC],
                                        smallp_sb[:, 2:3], None, op0=OP.mult)
                tot = sp.tile([P, MC], bf16, tag="a")
                nc.vector.tensor_tensor(tot[:, :], ys[:, :], skip[:, :], op=OP.add)
                y_t = sp.tile([P, MC], bf16, tag="b")
                nc.vector.tensor_tensor(y_t[:, :], tot[:, :],
                                        zg_sb[:, lo : lo + MC], op=OP.mult)
                if debug:
                    nc.sync.dma_start(dbg_y[:, lo : lo + MC], y_t[:, :])
                # out_proj partials for this block; RS per 1024-row half
                for st in range(MC // 128):
                    po = psD.tile([128, DM], f32, tag="po")
                    nc.tensor.matmul(po[:, :], lhsT=y_t[:, st * 128 : (st + 1) * 128],
                                     rhs=wout_sb[:, :], start=True, stop=True)
                    ob = wp.tile([128, DM], bf16, tag="ob")
                    nc.scalar.activation(ob[:, :], po[:, :], AF.Copy)
                    nc.sync.dma_start(op_in[b][st * 128 : (st + 1) * 128, :], ob[:, :])
                    if st % 8 == 7:
                        half = st // 8
                        q = 2 * b + half
                        nc.gpsimd.collective_compute(
                            "ReduceScatter", OP.add, replica_groups=rg,
                            ins=[op_in[b][half * 1024 : (half + 1) * 1024, :].opt()],
                            outs=[op_out[q].ap().opt()])
                        emit_ln(q)

            # ---- main schedule: lag-2 pipeline; dt(b+1) emitted mid-scan(b) ----
            for c in range(NQ):
                if c >= 2:
                    emit_scan(c - 2, dt_for=c - 1)
                emit_inproj(c)
                if c == 1:
                    emit_dt(0)
            emit_scan(NQ - 2, dt_for=NQ - 1)
            emit_scan(NQ - 1)

            if debug:
                nc.sync.dma_start(dbg_ua[:, :], ua_sb[:, :])
                nc.sync.dma_start(dbg_dt[:, :], dt_sb[:, :])
                nc.sync.dma_start(dbg_zg[:, :], zg_sb[:, :])

    # Restrict ACT tables to the two we use so the load-insertion pass
    # inserts switches only at clean phase boundaries.
    import concourse.bacc as bacc_mod
    orig_tables = bacc_mod.get_activation_tables
    KEEP = ("natural_log_exp_and_others", "silu_and_others")

    def _two_tables(arch):
        t = orig_tables(arch)
        return {k: (v if k in KEEP else set()) for k, v in t.items()}

    bacc_mod.get_activation_tables = _two_tables
    try:
        nc.compile()
    finally:
        bacc_mod.get_activation_tables = orig_tables
    return nc


def _get_nc():
    if "nc" not in _CACHE:
        _CACHE["nc"] = _build()
    return _CACHE["nc"]


def _core_rows(i):
    """Absolute check-node indices held by core i's output, in output order."""
    return np.concatenate(
        [np.arange(q * 1024 + i * 128, q * 1024 + (i + 1) * 128)
         for q in range(2 * NQ)])


def _make_in_maps(inputs):
    x = np.ascontiguousarray(np.asarray(inputs["x"], dtype=np.float32))
    seq_idx = np.asarray(inputs["seq_idx"], dtype=np.int64)
    W_in = np.asarray(inputs["W_in"], dtype=np.float32)
    conv_w = np.asarray(inputs["conv_w"], dtype=np.float32)
    conv_b = np.asarray(inputs["conv_b"], dtype=np.float32)
    W_xproj = np.asarray(inputs["W_xproj"], dtype=np.float32)
    W_dt = np.asarray(inputs["W_dt"], dtype=np.float32)
    b_dt = np.asarray(inputs["b_dt"], dtype=np.float32)
    A_log = np.asarray(inputs["A_log"], dtype=np.float32)
    D_skip = np.asarray(inputs["D_skip"], dtype=np.float32)
    W_out = np.asarray(inputs["W_out"], dtype=np.float32)
    ln_w = np.asarray(inputs["ln_w"], dtype=np.float32)
    ln_b = np.asarray(inputs["ln_b"], dtype=np.float32)

    # ln scale/bias are identity by construction; they are baked into the graph.
    assert np.allclose(ln_w, 1.0) and np.allclose(ln_b, 0.0), "non-identity LN params unsupported"

    x_chk = x[seq_idx]                               # [S, DM]
    xT = np.ascontiguousarray(x_chk.T).astype(BF16)  # [DM, S]

    in_maps = []
    for i in range(NCORE):
        cs = slice(i * P, (i + 1) * P)
        # conv folded into in_proj: W_k[:, d] = W_in[:, cs][:, d] * conv_w[cs][d, k]
        Wu = W_in[:, cs]                                       # [DM, P]
        cw = conv_w[cs]                                        # [P, DC]
        # wuc layout [p, kk, k, d]: p = dm % 128, kk = dm // 128
        wuc = np.empty((128, 4, DC, P), np.float32)
        for kk in range(4):
            blk = Wu[kk * 128 : (kk + 1) * 128, :]             # [128, P]
            for k in range(DC):
                wuc[:, kk, k, :] = blk * cw[None, :, k]
        wz = np.ascontiguousarray(
            W_in[:, DI + i * P : DI + (i + 1) * P].reshape(4, 128, P).transpose(1, 0, 2))
        in_maps.append({
            "xT": xT,
            "wuc": np.ascontiguousarray(wuc.reshape(128, 4 * DC * P)).astype(BF16),
            "wz": np.ascontiguousarray(wz.reshape(128, 4 * P)).astype(BF16),
            "wxp": np.ascontiguousarray(W_xproj[cs]).astype(BF16),
            "wdt": np.ascontiguousarray(W_dt[:, cs]).astype(BF16),
            "wout": np.ascontiguousarray(W_out[cs]).astype(BF16),
            "smallp": np.ascontiguousarray(
                np.stack([conv_b[cs], b_dt[cs], D_skip[cs], -conv_b[cs]],
                         axis=1).astype(np.float32)),
            "alog": np.ascontiguousarray(A_log[cs]),
            "xres": np.ascontiguousarray(x_chk[_core_rows(i)]),
        })
    return x, seq_idx, in_maps


def kernel(**inputs):
    from concourse.bass_utils import run_bass_kernel_spmd

    x, seq_idx, in_maps = _make_in_maps(inputs)
    nc = _get_nc()
    trace = bool(int(os.environ.get("KERNEL_TRACE", "0")))
    res = run_bass_kernel_spmd(nc, in_maps, core_ids=list(range(NCORE)), trace=trace)
    if trace:
        _CACHE["last_exec_time_ns"] = res.exec_time_ns
        _CACHE["last_results"] = res
    y = np.empty((S, DM), np.float32)
    for i in range(NCORE):
        y[_core_rows(i)] = np.asarray(res.results[i]["out"])
    outp = x.copy()
    outp[seq_idx] = y
    return outp
